# revision 19
# baseline (speedup 1.0000x reference)
"""Multi-head attention block (QKV proj + softmax attention + out proj) on 8
Trainium2 NeuronCores, data-parallel over the batch dimension (one batch
element per core).

Self-contained: hardcodes shapes for x [8, 1024, 768], qkv_w [768, 2304],
proj_w [768, 768], proj_b [768]; returns [8, 1024, 768] float32.
"""

import numpy as np

import concourse.bass as bass
import concourse.mybir as mybir
import concourse.tile as tile
from concourse import bacc

N_CORES = 8
N = 1024          # tokens per batch element
C = 768           # model dim
H = 12            # heads
HD = 64           # head dim
CT = C // 128     # 6 contraction tiles
TT = N // 128     # 8 token tiles
SCALE = HD ** -0.5

F32 = mybir.dt.float32

F32R = mybir.dt.float32r

# All matmul operands are float32r: 1 cycle/row (vs 4 for f32) when the
# moving dim >= 256. f32r = f32 with the low 11 mantissa bits zeroed; the
# host pre-rounds DMA-fed tensors, on-chip producers round on write.
QKV_MM = F32R
SCORE_MM = F32R
EXPV_MM = F32R
PROJ_MM = F32R


def _r(ap):
    return ap


def _build():
    nc = bacc.Bacc("TRN2", target_bir_lowering=False, debug=False,
                   num_devices=N_CORES)
    x_t = nc.dram_tensor("x_t", [C, N], QKV_MM, kind="ExternalInput").ap()
    qkv_w = nc.dram_tensor("qkv_w", [C, 3 * C], QKV_MM, kind="ExternalInput").ap()
    proj_w = nc.dram_tensor("proj_w", [C, C], PROJ_MM, kind="ExternalInput").ap()
    proj_b = nc.dram_tensor("proj_b", [1, C], PROJ_MM, kind="ExternalInput").ap()
    out = nc.dram_tensor("out", [N, C], F32, kind="ExternalOutput").ap()

    with tile.TileContext(nc) as tc:
        _emit(nc, tc, x_t, qkv_w, proj_w, proj_b, out)
    nc.compile()
    return nc


def _emit(nc, tc, x_t, qkv_w, proj_w, proj_b, out):
    from contextlib import ExitStack
    ctx = ExitStack()
    with ctx:
        wqk_pool = ctx.enter_context(tc.tile_pool(name="wqk", bufs=1))
        xt_pool = ctx.enter_context(tc.tile_pool(name="xt", bufs=1))
        w768_pool = ctx.enter_context(tc.tile_pool(name="w768", bufs=1))
        qk_pool = ctx.enter_context(tc.tile_pool(name="qk", bufs=1))
        vaug_pool = ctx.enter_context(tc.tile_pool(name="vaug", bufs=1))
        exps_pool = ctx.enter_context(tc.tile_pool(name="exps", bufs=6))
        misc_pool = ctx.enter_context(tc.tile_pool(name="misc", bufs=3))
        norm_pool = ctx.enter_context(tc.tile_pool(name="norm", bufs=2))
        const_pool = ctx.enter_context(tc.tile_pool(name="const", bufs=1))
        outsb_pool = ctx.enter_context(tc.tile_pool(name="outsb", bufs=2))
        dram_pool = ctx.enter_context(tc.tile_pool(name="drs", bufs=2, space="DRAM"))

        # ---- phase 0: loads ----
        XT = xt_pool.tile([128, CT, N], QKV_MM, tag="xt")
        for ct in range(CT):
            nc.sync.dma_start(XT[:, ct, :], x_t[ct * 128:(ct + 1) * 128, :])
        Wqk = wqk_pool.tile([128, CT, 2 * C], QKV_MM, tag="wqk")
        for ft in range(2 * CT):
            eng = nc.gpsimd if ft % 2 else nc.sync
            eng.dma_start(
                Wqk[:, :, ft * 128:(ft + 1) * 128],
                qkv_w[:, ft * 128:(ft + 1) * 128].rearrange("(c p) f -> p c f", p=128))
        Wv = w768_pool.tile([128, CT, C], QKV_MM, tag="w768")
        for ct in range(CT):
            nc.gpsimd.dma_start(Wv[:, ct, :], qkv_w[ct * 128:(ct + 1) * 128, 2 * C:3 * C])
        pb = const_pool.tile([1, C], PROJ_MM, tag="pb")
        nc.sync.dma_start(pb[:], proj_b[:, :])
        ones_st = const_pool.tile([128, 128], F32, tag="ones_st")
        nc.vector.memset(ones_st[:], 1.0)
        ones_r = const_pool.tile([1, 128], PROJ_MM, tag="ones")
        nc.vector.tensor_copy(ones_r[:], ones_st[0:1, :])

        # ---- phase 1: QKV ----
        QT = qk_pool.tile([128, CT, N], SCORE_MM, tag="qt")
        KT = qk_pool.tile([128, CT, N], SCORE_MM, tag="kt")
        V_AUG = vaug_pool.tile([128, TT, H, HD + 1], EXPV_MM, tag="vaug")
        nc.vector.tensor_copy(
            V_AUG[:, :, :, HD:HD + 1].rearrange("p t h one -> p (t h one)"),
            ones_st[:, 0:96])
        with tc.tile_pool(name="qkvps", bufs=4, space="PSUM") as qkv_ps:
            for ft in range(2 * CT):      # 0-5 -> Q^T rows, 6-11 -> K^T rows
                dest = QT if ft < CT else KT
                fi = ft % CT
                for qc in range(2):
                    ps = qkv_ps.tile([128, 512], F32, tag="qkvps")
                    for ct in range(CT):
                        nc.tensor.matmul(
                            ps[:],
                            lhsT=_r(Wqk[:, ct, ft * 128:(ft + 1) * 128]),
                            rhs=_r(XT[:, ct, qc * 512:(qc + 1) * 512]),
                            start=(ct == 0), stop=(ct == CT - 1))
                    nc.vector.tensor_copy(dest[:, fi, qc * 512:(qc + 1) * 512], ps[:])

            for tt in range(TT):
                for vc, (w0, wn, h0) in enumerate([(0, 512, 0), (512, 256, 8)]):
                    ps = qkv_ps.tile([128, 512], F32, tag="qkvps")
                    for ct in range(CT):
                        nc.tensor.matmul(
                            ps[:, :wn],
                            lhsT=_r(XT[:, ct, tt * 128:(tt + 1) * 128]),
                            rhs=_r(Wv[:, ct, w0:w0 + wn]),
                            start=(ct == 0), stop=(ct == CT - 1))
                    nc.vector.tensor_copy(
                        V_AUG[:, tt, h0:h0 + wn // HD, 0:HD],
                        ps[:, :wn].rearrange("p (h d) -> p h d", d=HD))

        PW = w768_pool.tile([128, CT, C], PROJ_MM, tag="w768")
        nc.sync.dma_start(PW[:], proj_w.rearrange("(c p) f -> p c f", p=128))

        # outT reuses the Wqk slot (Wqk is dead after QT/KT matmuls)
        outT = wqk_pool.tile([128, CT, N], PROJ_MM, tag="wqk")

        # ---- phase 2: attention per head-pair ----
        attn_ctx = ExitStack()
        sc_ps = attn_ctx.enter_context(tc.tile_pool(name="scps", bufs=2, space="PSUM"))
        av_ps = attn_ctx.enter_context(tc.tile_pool(name="avps", bufs=2, space="PSUM"))
        for hp in range(CT):
            avA = av_ps.tile([HD + 1, 1024], F32, tag="avps")
            avB = av_ps.tile([HD + 1, 1024], F32, tag="avps")

            def emit_av(kt, eA, eB, hp=hp, avA=avA, avB=avB):
                for qc in range(2):
                    nc.tensor.matmul(
                        avA[:, qc * 512:(qc + 1) * 512],
                        lhsT=_r(V_AUG[:, kt, 2 * hp, :]),
                        rhs=_r(eA[:, qc * 512:(qc + 1) * 512]),
                        start=(kt == 0), stop=(kt == TT - 1))
                    nc.tensor.matmul(
                        avB[:, qc * 512:(qc + 1) * 512],
                        lhsT=_r(V_AUG[:, kt, 2 * hp + 1, :]),
                        rhs=_r(eB[:, qc * 512:(qc + 1) * 512]),
                        start=(kt == 0), stop=(kt == TT - 1))

            pend = []
            for kt in range(TT):
                psA = sc_ps.tile([128, 1024], F32, tag="scps")
                psB = sc_ps.tile([128, 1024], F32, tag="scps")
                for qc in range(2):
                    nc.tensor.matmul(
                        psA[:, qc * 512:(qc + 1) * 512],
                        lhsT=_r(KT[0:64, hp, kt * 128:(kt + 1) * 128]),
                        rhs=_r(QT[0:64, hp, qc * 512:(qc + 1) * 512]),
                        start=True, stop=True)
                    nc.tensor.matmul(
                        psB[:, qc * 512:(qc + 1) * 512],
                        lhsT=_r(KT[64:128, hp, kt * 128:(kt + 1) * 128]),
                        rhs=_r(QT[64:128, hp, qc * 512:(qc + 1) * 512]),
                        start=True, stop=True)
                eA = exps_pool.tile([128, 1024], EXPV_MM, tag="exps")
                eB = exps_pool.tile([128, 1024], EXPV_MM, tag="exps")
                nc.scalar.activation(eA[:], psA[:], mybir.ActivationFunctionType.Exp,
                                     scale=SCALE)
                nc.scalar.activation(eB[:], psB[:], mybir.ActivationFunctionType.Exp,
                                     scale=SCALE)
                # delay AV emission one kt so the PE queue always holds the
                # next scores pair ahead of exp-dependent AV work
                pend.append((kt, eA, eB))
                if len(pend) >= 3:
                    emit_av(*pend.pop(0))
            for args in pend:
                emit_av(*args)
            for av, poff in ((avA, 0), (avB, 64)):
                # evacuate PSUM immediately so the next head-pair's AV can
                # allocate banks; the reciprocal chain runs from SBUF async
                U = norm_pool.tile([HD + 1, 1024], F32, tag="U")
                nc.vector.tensor_copy(U[:], av[:])
                dscr = dram_pool.tile([1024], F32, tag="dscr")
                nc.gpsimd.dma_start(dscr[:], U[HD:HD + 1, :])
                Dt = misc_pool.tile([64, 16], F32, tag="Dt")
                nc.gpsimd.dma_start(Dt[:], dscr[:].rearrange("(p j) -> p j", j=16))
                Rt = misc_pool.tile([64, 16], F32, tag="Rt")
                scr = misc_pool.tile([64, 16], F32, tag="scr")
                nc.vector.reciprocal_approx_accurate(Rt[:], Dt[:], scr[:])
                rscr = dram_pool.tile([1024], F32, tag="rscr")
                nc.gpsimd.dma_start(rscr[:].rearrange("(p j) -> p j", j=16), Rt[:])
                bc = norm_pool.tile([64, 1024], F32, tag="bc")
                rs = rscr[:]
                bcast_ap = bass.AP(tensor=rs.tensor, offset=rs.offset,
                                   ap=[[0, 64]] + [list(a) for a in rs.ap])
                nc.gpsimd.dma_start(bc[:], bcast_ap)
                for qc in range(2):
                    nc.vector.tensor_mul(
                        outT[poff:poff + 64, hp, qc * 512:(qc + 1) * 512],
                        U[0:HD, qc * 512:(qc + 1) * 512],
                        bc[:, qc * 512:(qc + 1) * 512])

        attn_ctx.close()

        # ---- phase 3: proj + bias ----
        pj_ctx = ExitStack()
        pj_ps = pj_ctx.enter_context(tc.tile_pool(name="pjps", bufs=4, space="PSUM"))
        for tt in range(TT):
            osb = outsb_pool.tile([128, C], F32, tag="outsb")
            for nch in range(2):
                ps = pj_ps.tile([128, 384], F32, tag="pjps")
                for ct in range(CT):
                    nc.tensor.matmul(
                        ps[:],
                        lhsT=_r(outT[:, ct, tt * 128:(tt + 1) * 128]),
                        rhs=_r(PW[:, ct, nch * 384:(nch + 1) * 384]),
                        start=(ct == 0), stop=False)
                nc.tensor.matmul(
                    ps[:],
                    lhsT=_r(ones_r[0:1, 0:128]),
                    rhs=_r(pb[0:1, nch * 384:(nch + 1) * 384]),
                    start=False, stop=True)
                nc.vector.tensor_copy(osb[:, nch * 384:(nch + 1) * 384], ps[:])
            nc.sync.dma_start(out[tt * 128:(tt + 1) * 128, :], osb[:])
        pj_ctx.close()


_CACHE = {}


def _get_runner():
    """Build + compile once; return a callable(in_maps) -> list of out dicts.

    Keeps a persistent jitted shard_map executable so repeat calls skip
    retracing/recompiling (mirrors bass2jax.run_bass_via_pjrt).
    """
    if "runner" in _CACHE:
        return _CACHE["runner"]

    import jax
    from jax.experimental.shard_map import shard_map
    from jax.sharding import Mesh, PartitionSpec
    from concourse import bass2jax

    nc = _build()
    bass2jax.install_neuronx_cc_hook()

    partition_name = (nc.partition_id_tensor.name if nc.partition_id_tensor
                      else None)
    in_names, out_names, out_avals, zero_outs = [], [], [], []
    for alloc in nc.m.functions[0].allocations:
        if not isinstance(alloc, mybir.MemoryLocationSet):
            continue
        name = alloc.memorylocations[0].name
        if alloc.kind == "ExternalInput":
            if name != partition_name:
                in_names.append(name)
        elif alloc.kind == "ExternalOutput":
            out_names.append(name)
            shape = tuple(alloc.tensor_shape)
            dtype = mybir.dt.np(alloc.dtype)
            out_avals.append(jax.core.ShapedArray(shape, dtype))
            zero_outs.append(np.zeros(shape, dtype))
    n_params = len(in_names)
    n_outs = len(out_avals)
    all_in_names = list(in_names) + list(out_names)
    if partition_name is not None:
        all_in_names.append(partition_name)
    donate = tuple(range(n_params, n_params + n_outs))

    def _body(*args):
        operands = list(args)
        if partition_name is not None:
            operands.append(bass2jax.partition_id_tensor())
        outs = bass2jax._bass_exec_p.bind(
            *operands,
            out_avals=tuple(out_avals),
            in_names=tuple(all_in_names),
            out_names=tuple(out_names),
            lowering_input_output_aliases=(),
            sim_require_finite=True,
            sim_require_nnan=True,
            nc=nc,
        )
        return tuple(outs)

    devices = jax.devices()[:N_CORES]
    mesh = Mesh(np.asarray(devices), ("core",))
    in_specs = (PartitionSpec("core"),) * (n_params + n_outs)
    out_specs = (PartitionSpec("core"),) * n_outs
    sharded = jax.jit(
        shard_map(_body, mesh=mesh, in_specs=in_specs, out_specs=out_specs,
                  check_rep=False),
        donate_argnums=donate, keep_unused=True)

    def runner(in_maps):
        concat_in = [
            np.concatenate([np.asarray(m[name]) for m in in_maps], axis=0)
            for name in in_names
        ]
        concat_zeros = [
            np.zeros((N_CORES * z.shape[0], *z.shape[1:]), z.dtype)
            for z in zero_outs
        ]
        out_arrs = sharded(*concat_in, *concat_zeros)
        return [
            {name: np.asarray(out_arrs[i]).reshape(N_CORES, *out_avals[i].shape)[c]
             for i, name in enumerate(out_names)}
            for c in range(N_CORES)
        ]

    _CACHE["runner"] = runner
    _CACHE["nc"] = nc
    return runner


def _round_f32r(a):
    a = np.ascontiguousarray(a, dtype=np.float32)
    b = a.view(np.uint32) & np.uint32(0xFFFFF800)
    return b.view(np.float32)


def make_in_maps(x, qkv_w, proj_w, proj_b):
    qkv_w = _round_f32r(qkv_w)
    proj_w = _round_f32r(proj_w)
    pb = _round_f32r(np.asarray(proj_b).reshape(1, C))
    return [
        {
            "x_t": _round_f32r(np.asarray(x[b], dtype=np.float32).T),
            "qkv_w": qkv_w,
            "proj_w": proj_w,
            "proj_b": pb,
        }
        for b in range(N_CORES)
    ]


def kernel(x, qkv_w, proj_w, proj_b):
    runner = _get_runner()
    results = runner(make_in_maps(x, qkv_w, proj_w, proj_b))
    return np.stack([results[b]["out"] for b in range(N_CORES)], axis=0)


# revision 20
# speedup vs baseline: 1.0028x; 1.0028x over previous
"""Multi-head attention block (QKV proj + softmax attention + out proj) on 8
Trainium2 NeuronCores, data-parallel over the batch dimension (one batch
element per core).

Self-contained: hardcodes shapes for x [8, 1024, 768], qkv_w [768, 2304],
proj_w [768, 768], proj_b [768]; returns [8, 1024, 768] float32.
"""

import numpy as np

import concourse.bass as bass
import concourse.mybir as mybir
import concourse.tile as tile
from concourse import bacc

N_CORES = 8
N = 1024          # tokens per batch element
C = 768           # model dim
H = 12            # heads
HD = 64           # head dim
CT = C // 128     # 6 contraction tiles
TT = N // 128     # 8 token tiles
SCALE = HD ** -0.5

F32 = mybir.dt.float32

F32R = mybir.dt.float32r

# All matmul operands are float32r: 1 cycle/row (vs 4 for f32) when the
# moving dim >= 256. f32r = f32 with the low 11 mantissa bits zeroed; the
# host pre-rounds DMA-fed tensors, on-chip producers round on write.
QKV_MM = F32R
SCORE_MM = F32R
EXPV_MM = F32R
PROJ_MM = F32R


def _r(ap):
    return ap


def _build():
    nc = bacc.Bacc("TRN2", target_bir_lowering=False, debug=False,
                   num_devices=N_CORES)
    x_t = nc.dram_tensor("x_t", [C, N], QKV_MM, kind="ExternalInput").ap()
    qkv_w = nc.dram_tensor("qkv_w", [C, 3 * C], QKV_MM, kind="ExternalInput").ap()
    proj_w = nc.dram_tensor("proj_w", [C, C], PROJ_MM, kind="ExternalInput").ap()
    proj_b = nc.dram_tensor("proj_b", [1, C], PROJ_MM, kind="ExternalInput").ap()
    out = nc.dram_tensor("out", [N, C], F32, kind="ExternalOutput").ap()

    with tile.TileContext(nc) as tc:
        _emit(nc, tc, x_t, qkv_w, proj_w, proj_b, out)
    nc.compile()
    return nc


def _emit(nc, tc, x_t, qkv_w, proj_w, proj_b, out):
    from contextlib import ExitStack
    ctx = ExitStack()
    with ctx:
        wqk_pool = ctx.enter_context(tc.tile_pool(name="wqk", bufs=1))
        xt_pool = ctx.enter_context(tc.tile_pool(name="xt", bufs=1))
        w768_pool = ctx.enter_context(tc.tile_pool(name="w768", bufs=1))
        qk_pool = ctx.enter_context(tc.tile_pool(name="qk", bufs=1))
        vaug_pool = ctx.enter_context(tc.tile_pool(name="vaug", bufs=1))
        exps_pool = ctx.enter_context(tc.tile_pool(name="exps", bufs=6))
        misc_pool = ctx.enter_context(tc.tile_pool(name="misc", bufs=3))
        norm_pool = ctx.enter_context(tc.tile_pool(name="norm", bufs=2))
        const_pool = ctx.enter_context(tc.tile_pool(name="const", bufs=1))
        outsb_pool = ctx.enter_context(tc.tile_pool(name="outsb", bufs=2))
        dram_pool = ctx.enter_context(tc.tile_pool(name="drs", bufs=2, space="DRAM"))

        # ---- phase 0: loads ----
        XT = xt_pool.tile([128, CT, N], QKV_MM, tag="xt")
        for ct in range(CT):
            nc.sync.dma_start(XT[:, ct, :], x_t[ct * 128:(ct + 1) * 128, :])
        Wqk = wqk_pool.tile([128, CT, 2 * C], QKV_MM, tag="wqk")
        for ft in range(2 * CT):
            nc.sync.dma_start(
                Wqk[:, :, ft * 128:(ft + 1) * 128],
                qkv_w[:, ft * 128:(ft + 1) * 128].rearrange("(c p) f -> p c f", p=128))
        Wv = w768_pool.tile([128, CT, C], QKV_MM, tag="w768")
        for ct in range(CT):
            nc.sync.dma_start(Wv[:, ct, :], qkv_w[ct * 128:(ct + 1) * 128, 2 * C:3 * C])
        pb = const_pool.tile([1, C], PROJ_MM, tag="pb")
        nc.sync.dma_start(pb[:], proj_b[:, :])
        ones_st = const_pool.tile([128, 128], F32, tag="ones_st")
        nc.vector.memset(ones_st[:], 1.0)
        ones_r = const_pool.tile([1, 128], PROJ_MM, tag="ones")
        nc.vector.tensor_copy(ones_r[:], ones_st[0:1, :])

        # ---- phase 1: QKV ----
        QT = qk_pool.tile([128, CT, N], SCORE_MM, tag="qt")
        KT = qk_pool.tile([128, CT, N], SCORE_MM, tag="kt")
        V_AUG = vaug_pool.tile([128, TT, H, HD + 1], EXPV_MM, tag="vaug")
        nc.vector.tensor_copy(
            V_AUG[:, :, :, HD:HD + 1].rearrange("p t h one -> p (t h one)"),
            ones_st[:, 0:96])
        with tc.tile_pool(name="qkvps", bufs=4, space="PSUM") as qkv_ps:
            for ft in range(2 * CT):      # 0-5 -> Q^T rows, 6-11 -> K^T rows
                dest = QT if ft < CT else KT
                fi = ft % CT
                for qc in range(2):
                    ps = qkv_ps.tile([128, 512], F32, tag="qkvps")
                    for ct in range(CT):
                        nc.tensor.matmul(
                            ps[:],
                            lhsT=_r(Wqk[:, ct, ft * 128:(ft + 1) * 128]),
                            rhs=_r(XT[:, ct, qc * 512:(qc + 1) * 512]),
                            start=(ct == 0), stop=(ct == CT - 1))
                    nc.vector.tensor_copy(dest[:, fi, qc * 512:(qc + 1) * 512], ps[:])

            for tt in range(TT):
                for vc, (w0, wn, h0) in enumerate([(0, 512, 0), (512, 256, 8)]):
                    ps = qkv_ps.tile([128, 512], F32, tag="qkvps")
                    for ct in range(CT):
                        nc.tensor.matmul(
                            ps[:, :wn],
                            lhsT=_r(XT[:, ct, tt * 128:(tt + 1) * 128]),
                            rhs=_r(Wv[:, ct, w0:w0 + wn]),
                            start=(ct == 0), stop=(ct == CT - 1))
                    nc.vector.tensor_copy(
                        V_AUG[:, tt, h0:h0 + wn // HD, 0:HD],
                        ps[:, :wn].rearrange("p (h d) -> p h d", d=HD))

        PW = w768_pool.tile([128, CT, C], PROJ_MM, tag="w768")
        nc.sync.dma_start(PW[:], proj_w.rearrange("(c p) f -> p c f", p=128))

        # outT reuses the Wqk slot (Wqk is dead after QT/KT matmuls)
        outT = wqk_pool.tile([128, CT, N], PROJ_MM, tag="wqk")

        # ---- phase 2: attention per head-pair ----
        attn_ctx = ExitStack()
        sc_ps = attn_ctx.enter_context(tc.tile_pool(name="scps", bufs=2, space="PSUM"))
        av_ps = attn_ctx.enter_context(tc.tile_pool(name="avps", bufs=2, space="PSUM"))
        for hp in range(CT):
            avA = av_ps.tile([HD + 1, 1024], F32, tag="avps")
            avB = av_ps.tile([HD + 1, 1024], F32, tag="avps")

            def emit_av(kt, eA, eB, hp=hp, avA=avA, avB=avB):
                for qc in range(2):
                    nc.tensor.matmul(
                        avA[:, qc * 512:(qc + 1) * 512],
                        lhsT=_r(V_AUG[:, kt, 2 * hp, :]),
                        rhs=_r(eA[:, qc * 512:(qc + 1) * 512]),
                        start=(kt == 0), stop=(kt == TT - 1))
                    nc.tensor.matmul(
                        avB[:, qc * 512:(qc + 1) * 512],
                        lhsT=_r(V_AUG[:, kt, 2 * hp + 1, :]),
                        rhs=_r(eB[:, qc * 512:(qc + 1) * 512]),
                        start=(kt == 0), stop=(kt == TT - 1))

            pend = []
            for kt in range(TT):
                psA = sc_ps.tile([128, 1024], F32, tag="scps")
                psB = sc_ps.tile([128, 1024], F32, tag="scps")
                for qc in range(2):
                    nc.tensor.matmul(
                        psA[:, qc * 512:(qc + 1) * 512],
                        lhsT=_r(KT[0:64, hp, kt * 128:(kt + 1) * 128]),
                        rhs=_r(QT[0:64, hp, qc * 512:(qc + 1) * 512]),
                        start=True, stop=True)
                    nc.tensor.matmul(
                        psB[:, qc * 512:(qc + 1) * 512],
                        lhsT=_r(KT[64:128, hp, kt * 128:(kt + 1) * 128]),
                        rhs=_r(QT[64:128, hp, qc * 512:(qc + 1) * 512]),
                        start=True, stop=True)
                eA = exps_pool.tile([128, 1024], EXPV_MM, tag="exps")
                eB = exps_pool.tile([128, 1024], EXPV_MM, tag="exps")
                nc.scalar.activation(eA[:], psA[:], mybir.ActivationFunctionType.Exp,
                                     scale=SCALE)
                nc.scalar.activation(eB[:], psB[:], mybir.ActivationFunctionType.Exp,
                                     scale=SCALE)
                # delay AV emission one kt so the PE queue always holds the
                # next scores pair ahead of exp-dependent AV work
                pend.append((kt, eA, eB))
                if len(pend) >= 3:
                    emit_av(*pend.pop(0))
            for args in pend:
                emit_av(*args)
            for av, poff in ((avA, 0), (avB, 64)):
                # evacuate PSUM immediately so the next head-pair's AV can
                # allocate banks; the reciprocal chain runs from SBUF async
                U = norm_pool.tile([HD + 1, 1024], F32, tag="U")
                nc.vector.tensor_copy(U[:], av[:])
                dscr = dram_pool.tile([1024], F32, tag="dscr")
                nc.gpsimd.dma_start(dscr[:], U[HD:HD + 1, :])
                Dt = misc_pool.tile([64, 16], F32, tag="Dt")
                nc.gpsimd.dma_start(Dt[:], dscr[:].rearrange("(p j) -> p j", j=16))
                Rt = misc_pool.tile([64, 16], F32, tag="Rt")
                scr = misc_pool.tile([64, 16], F32, tag="scr")
                nc.vector.reciprocal_approx_accurate(Rt[:], Dt[:], scr[:])
                rscr = dram_pool.tile([1024], F32, tag="rscr")
                nc.gpsimd.dma_start(rscr[:].rearrange("(p j) -> p j", j=16), Rt[:])
                bc = norm_pool.tile([64, 1024], F32, tag="bc")
                rs = rscr[:]
                bcast_ap = bass.AP(tensor=rs.tensor, offset=rs.offset,
                                   ap=[[0, 64]] + [list(a) for a in rs.ap])
                nc.gpsimd.dma_start(bc[:], bcast_ap)
                for qc in range(2):
                    nc.vector.tensor_mul(
                        outT[poff:poff + 64, hp, qc * 512:(qc + 1) * 512],
                        U[0:HD, qc * 512:(qc + 1) * 512],
                        bc[:, qc * 512:(qc + 1) * 512])

        attn_ctx.close()

        # ---- phase 3: proj + bias ----
        pj_ctx = ExitStack()
        pj_ps = pj_ctx.enter_context(tc.tile_pool(name="pjps", bufs=4, space="PSUM"))
        for tt in range(TT):
            osb = outsb_pool.tile([128, C], F32, tag="outsb")
            for nch in range(2):
                ps = pj_ps.tile([128, 384], F32, tag="pjps")
                for ct in range(CT):
                    nc.tensor.matmul(
                        ps[:],
                        lhsT=_r(outT[:, ct, tt * 128:(tt + 1) * 128]),
                        rhs=_r(PW[:, ct, nch * 384:(nch + 1) * 384]),
                        start=(ct == 0), stop=False)
                nc.tensor.matmul(
                    ps[:],
                    lhsT=_r(ones_r[0:1, 0:128]),
                    rhs=_r(pb[0:1, nch * 384:(nch + 1) * 384]),
                    start=False, stop=True)
                nc.vector.tensor_copy(osb[:, nch * 384:(nch + 1) * 384], ps[:])
            nc.sync.dma_start(out[tt * 128:(tt + 1) * 128, :], osb[:])
        pj_ctx.close()


_CACHE = {}


def _get_runner():
    """Build + compile once; return a callable(in_maps) -> list of out dicts.

    Keeps a persistent jitted shard_map executable so repeat calls skip
    retracing/recompiling (mirrors bass2jax.run_bass_via_pjrt).
    """
    if "runner" in _CACHE:
        return _CACHE["runner"]

    import jax
    from jax.experimental.shard_map import shard_map
    from jax.sharding import Mesh, PartitionSpec
    from concourse import bass2jax

    nc = _build()
    bass2jax.install_neuronx_cc_hook()

    partition_name = (nc.partition_id_tensor.name if nc.partition_id_tensor
                      else None)
    in_names, out_names, out_avals, zero_outs = [], [], [], []
    for alloc in nc.m.functions[0].allocations:
        if not isinstance(alloc, mybir.MemoryLocationSet):
            continue
        name = alloc.memorylocations[0].name
        if alloc.kind == "ExternalInput":
            if name != partition_name:
                in_names.append(name)
        elif alloc.kind == "ExternalOutput":
            out_names.append(name)
            shape = tuple(alloc.tensor_shape)
            dtype = mybir.dt.np(alloc.dtype)
            out_avals.append(jax.core.ShapedArray(shape, dtype))
            zero_outs.append(np.zeros(shape, dtype))
    n_params = len(in_names)
    n_outs = len(out_avals)
    all_in_names = list(in_names) + list(out_names)
    if partition_name is not None:
        all_in_names.append(partition_name)
    donate = tuple(range(n_params, n_params + n_outs))

    def _body(*args):
        operands = list(args)
        if partition_name is not None:
            operands.append(bass2jax.partition_id_tensor())
        outs = bass2jax._bass_exec_p.bind(
            *operands,
            out_avals=tuple(out_avals),
            in_names=tuple(all_in_names),
            out_names=tuple(out_names),
            lowering_input_output_aliases=(),
            sim_require_finite=True,
            sim_require_nnan=True,
            nc=nc,
        )
        return tuple(outs)

    devices = jax.devices()[:N_CORES]
    mesh = Mesh(np.asarray(devices), ("core",))
    in_specs = (PartitionSpec("core"),) * (n_params + n_outs)
    out_specs = (PartitionSpec("core"),) * n_outs
    sharded = jax.jit(
        shard_map(_body, mesh=mesh, in_specs=in_specs, out_specs=out_specs,
                  check_rep=False),
        donate_argnums=donate, keep_unused=True)

    def runner(in_maps):
        concat_in = [
            np.concatenate([np.asarray(m[name]) for m in in_maps], axis=0)
            for name in in_names
        ]
        concat_zeros = [
            np.zeros((N_CORES * z.shape[0], *z.shape[1:]), z.dtype)
            for z in zero_outs
        ]
        out_arrs = sharded(*concat_in, *concat_zeros)
        return [
            {name: np.asarray(out_arrs[i]).reshape(N_CORES, *out_avals[i].shape)[c]
             for i, name in enumerate(out_names)}
            for c in range(N_CORES)
        ]

    _CACHE["runner"] = runner
    _CACHE["nc"] = nc
    return runner


def _round_f32r(a):
    a = np.ascontiguousarray(a, dtype=np.float32)
    b = a.view(np.uint32) & np.uint32(0xFFFFF800)
    return b.view(np.float32)


def make_in_maps(x, qkv_w, proj_w, proj_b):
    qkv_w = _round_f32r(qkv_w)
    proj_w = _round_f32r(proj_w)
    pb = _round_f32r(np.asarray(proj_b).reshape(1, C))
    return [
        {
            "x_t": _round_f32r(np.asarray(x[b], dtype=np.float32).T),
            "qkv_w": qkv_w,
            "proj_w": proj_w,
            "proj_b": pb,
        }
        for b in range(N_CORES)
    ]


def kernel(x, qkv_w, proj_w, proj_b):
    runner = _get_runner()
    results = runner(make_in_maps(x, qkv_w, proj_w, proj_b))
    return np.stack([results[b]["out"] for b in range(N_CORES)], axis=0)


# revision 21
# speedup vs baseline: 1.0826x; 1.0796x over previous
"""Multi-head attention block (QKV proj + softmax attention + out proj) on 8
Trainium2 NeuronCores, data-parallel over the batch dimension (one batch
element per core).

Self-contained: hardcodes shapes for x [8, 1024, 768], qkv_w [768, 2304],
proj_w [768, 768], proj_b [768]; returns [8, 1024, 768] float32.
"""

import numpy as np

import concourse.bass as bass
import concourse.mybir as mybir
import concourse.tile as tile
from concourse import bacc

N_CORES = 8
N = 1024          # tokens per batch element
C = 768           # model dim
H = 12            # heads
HD = 64           # head dim
CT = C // 128     # 6 contraction tiles
TT = N // 128     # 8 token tiles
SCALE = HD ** -0.5

F32 = mybir.dt.float32

F32R = mybir.dt.float32r

# All matmul operands are float32r: 1 cycle/row (vs 4 for f32) when the
# moving dim >= 256. f32r = f32 with the low 11 mantissa bits zeroed; the
# host pre-rounds DMA-fed tensors, on-chip producers round on write.
QKV_MM = F32R
SCORE_MM = F32R
EXPV_MM = F32R
PROJ_MM = F32R


def _r(ap):
    return ap


def _build():
    nc = bacc.Bacc("TRN2", target_bir_lowering=False, debug=False,
                   num_devices=N_CORES)
    x_t = nc.dram_tensor("x_t", [C, N], QKV_MM, kind="ExternalInput").ap()
    qkv_w = nc.dram_tensor("qkv_w", [C, 3 * C], QKV_MM, kind="ExternalInput").ap()
    proj_w = nc.dram_tensor("proj_w", [C, C], PROJ_MM, kind="ExternalInput").ap()
    proj_b = nc.dram_tensor("proj_b", [1, C], PROJ_MM, kind="ExternalInput").ap()
    out = nc.dram_tensor("out", [N, C], F32, kind="ExternalOutput").ap()

    with tile.TileContext(nc) as tc:
        _emit(nc, tc, x_t, qkv_w, proj_w, proj_b, out)
    nc.compile()
    return nc


def _emit(nc, tc, x_t, qkv_w, proj_w, proj_b, out):
    from contextlib import ExitStack
    ctx = ExitStack()
    with ctx:
        wqk_pool = ctx.enter_context(tc.tile_pool(name="wqk", bufs=1))
        xt_pool = ctx.enter_context(tc.tile_pool(name="xt", bufs=1))
        w768_pool = ctx.enter_context(tc.tile_pool(name="w768", bufs=1))
        qk_pool = ctx.enter_context(tc.tile_pool(name="qk", bufs=1))
        vaug_pool = ctx.enter_context(tc.tile_pool(name="vaug", bufs=1))
        exps_pool = ctx.enter_context(tc.tile_pool(name="exps", bufs=6))
        misc_pool = ctx.enter_context(tc.tile_pool(name="misc", bufs=3))
        norm_pool = ctx.enter_context(tc.tile_pool(name="norm", bufs=2))
        const_pool = ctx.enter_context(tc.tile_pool(name="const", bufs=1))
        outsb_pool = ctx.enter_context(tc.tile_pool(name="outsb", bufs=2))
        dram_pool = ctx.enter_context(tc.tile_pool(name="drs", bufs=2, space="DRAM"))

        # ---- phase 0: loads ----
        XT = xt_pool.tile([128, CT, N], QKV_MM, tag="xt")
        Wv = w768_pool.tile([128, CT, C], QKV_MM, tag="w768")
        for ct in range(CT):
            nc.sync.dma_start(Wv[:, ct, :], qkv_w[ct * 128:(ct + 1) * 128, 2 * C:3 * C])
        for tt in range(TT):
            # token-chunked so the V matmuls for chunk tt start as soon as it lands
            nc.sync.dma_start(
                XT[:, :, tt * 128:(tt + 1) * 128],
                x_t[:, tt * 128:(tt + 1) * 128].rearrange("(c p) n -> p c n", p=128))
        Wqk = wqk_pool.tile([128, CT, 2 * C], QKV_MM, tag="wqk")
        for ft in range(2 * CT):
            nc.sync.dma_start(
                Wqk[:, :, ft * 128:(ft + 1) * 128],
                qkv_w[:, ft * 128:(ft + 1) * 128].rearrange("(c p) f -> p c f", p=128))
        pbb = const_pool.tile([128, C], PROJ_MM, tag="pb")
        pb_src = proj_b[:, :]
        pb_bcast = bass.AP(tensor=pb_src.tensor, offset=pb_src.offset,
                           ap=[[0, 128]] + [list(a) for a in pb_src.ap[1:]])
        nc.sync.dma_start(pbb[:], pb_bcast)
        ones_st = const_pool.tile([128, 128], F32, tag="ones_st")
        nc.vector.memset(ones_st[:], 1.0)

        # ---- phase 1: QKV ----
        QT = qk_pool.tile([128, CT, N], SCORE_MM, tag="qt")
        KT = qk_pool.tile([128, CT, N], SCORE_MM, tag="kt")
        V_AUG = vaug_pool.tile([128, TT, H, HD + 1], EXPV_MM, tag="vaug")
        nc.vector.tensor_copy(
            V_AUG[:, :, :, HD:HD + 1].rearrange("p t h one -> p (t h one)"),
            ones_st[:, 0:96])
        with tc.tile_pool(name="qkvps", bufs=4, space="PSUM") as qkv_ps:
            for tt in range(TT):
                for vc, (w0, wn, h0) in enumerate([(0, 512, 0), (512, 256, 8)]):
                    ps = qkv_ps.tile([128, 512], F32, tag="qkvps")
                    for ct in range(CT):
                        nc.tensor.matmul(
                            ps[:, :wn],
                            lhsT=_r(XT[:, ct, tt * 128:(tt + 1) * 128]),
                            rhs=_r(Wv[:, ct, w0:w0 + wn]),
                            start=(ct == 0), stop=(ct == CT - 1))
                    nc.vector.tensor_copy(
                        V_AUG[:, tt, h0:h0 + wn // HD, 0:HD],
                        ps[:, :wn].rearrange("p (h d) -> p h d", d=HD))

            for ft in range(2 * CT):      # 0-5 -> Q^T rows, 6-11 -> K^T rows
                dest = QT if ft < CT else KT
                fi = ft % CT
                for qc in range(2):
                    ps = qkv_ps.tile([128, 512], F32, tag="qkvps")
                    for ct in range(CT):
                        nc.tensor.matmul(
                            ps[:],
                            lhsT=_r(Wqk[:, ct, ft * 128:(ft + 1) * 128]),
                            rhs=_r(XT[:, ct, qc * 512:(qc + 1) * 512]),
                            start=(ct == 0), stop=(ct == CT - 1))
                    nc.vector.tensor_copy(dest[:, fi, qc * 512:(qc + 1) * 512], ps[:])

        PW = w768_pool.tile([128, CT, C], PROJ_MM, tag="w768")
        nc.sync.dma_start(PW[:], proj_w.rearrange("(c p) f -> p c f", p=128))

        # outT reuses the Wqk slot (Wqk is dead after QT/KT matmuls)
        outT = wqk_pool.tile([128, CT, N], PROJ_MM, tag="wqk")

        # ---- phase 2: attention per head-pair ----
        attn_ctx = ExitStack()
        sc_ps = attn_ctx.enter_context(tc.tile_pool(name="scps", bufs=2, space="PSUM"))
        av_ps = attn_ctx.enter_context(tc.tile_pool(name="avps", bufs=2, space="PSUM"))
        for hp in range(CT):
            avA = av_ps.tile([HD + 1, 1024], F32, tag="avps")
            avB = av_ps.tile([HD + 1, 1024], F32, tag="avps")

            def emit_av(kt, eA, eB, hp=hp, avA=avA, avB=avB):
                for qc in range(2):
                    nc.tensor.matmul(
                        avA[:, qc * 512:(qc + 1) * 512],
                        lhsT=_r(V_AUG[:, kt, 2 * hp, :]),
                        rhs=_r(eA[:, qc * 512:(qc + 1) * 512]),
                        start=(kt == 0), stop=(kt == TT - 1))
                    nc.tensor.matmul(
                        avB[:, qc * 512:(qc + 1) * 512],
                        lhsT=_r(V_AUG[:, kt, 2 * hp + 1, :]),
                        rhs=_r(eB[:, qc * 512:(qc + 1) * 512]),
                        start=(kt == 0), stop=(kt == TT - 1))

            pend = []
            for kt in range(TT):
                psA = sc_ps.tile([128, 1024], F32, tag="scps")
                psB = sc_ps.tile([128, 1024], F32, tag="scps")
                for qc in range(2):
                    nc.tensor.matmul(
                        psA[:, qc * 512:(qc + 1) * 512],
                        lhsT=_r(KT[0:64, hp, kt * 128:(kt + 1) * 128]),
                        rhs=_r(QT[0:64, hp, qc * 512:(qc + 1) * 512]),
                        start=True, stop=True)
                    nc.tensor.matmul(
                        psB[:, qc * 512:(qc + 1) * 512],
                        lhsT=_r(KT[64:128, hp, kt * 128:(kt + 1) * 128]),
                        rhs=_r(QT[64:128, hp, qc * 512:(qc + 1) * 512]),
                        start=True, stop=True)
                eA = exps_pool.tile([128, 1024], EXPV_MM, tag="exps")
                eB = exps_pool.tile([128, 1024], EXPV_MM, tag="exps")
                nc.scalar.activation(eA[:], psA[:], mybir.ActivationFunctionType.Exp,
                                     scale=SCALE)
                nc.scalar.activation(eB[:], psB[:], mybir.ActivationFunctionType.Exp,
                                     scale=SCALE)
                # delay AV emission one kt so the PE queue always holds the
                # next scores pair ahead of exp-dependent AV work
                pend.append((kt, eA, eB))
                if len(pend) >= 3:
                    emit_av(*pend.pop(0))
            for args in pend:
                emit_av(*args)
            for av, poff in ((avA, 0), (avB, 64)):
                # evacuate PSUM immediately so the next head-pair's AV can
                # allocate banks; the reciprocal chain runs from SBUF async
                U = norm_pool.tile([HD + 1, 1024], F32, tag="U")
                nc.vector.tensor_copy(U[:], av[:])
                dscr = dram_pool.tile([1024], F32, tag="dscr")
                nc.gpsimd.dma_start(dscr[:], U[HD:HD + 1, :])
                Dt = misc_pool.tile([64, 16], F32, tag="Dt")
                nc.gpsimd.dma_start(Dt[:], dscr[:].rearrange("(p j) -> p j", j=16))
                Rt = misc_pool.tile([64, 16], F32, tag="Rt")
                scr = misc_pool.tile([64, 16], F32, tag="scr")
                nc.vector.reciprocal_approx_accurate(Rt[:], Dt[:], scr[:])
                rscr = dram_pool.tile([1024], F32, tag="rscr")
                nc.gpsimd.dma_start(rscr[:].rearrange("(p j) -> p j", j=16), Rt[:])
                bc = norm_pool.tile([64, 1024], F32, tag="bc")
                rs = rscr[:]
                bcast_ap = bass.AP(tensor=rs.tensor, offset=rs.offset,
                                   ap=[[0, 64]] + [list(a) for a in rs.ap])
                nc.gpsimd.dma_start(bc[:], bcast_ap)
                for qc in range(2):
                    nc.vector.tensor_mul(
                        outT[poff:poff + 64, hp, qc * 512:(qc + 1) * 512],
                        U[0:HD, qc * 512:(qc + 1) * 512],
                        bc[:, qc * 512:(qc + 1) * 512])

        attn_ctx.close()

        # ---- phase 3: proj + bias ----
        pj_ctx = ExitStack()
        pj_ps = pj_ctx.enter_context(tc.tile_pool(name="pjps", bufs=4, space="PSUM"))
        for tt in range(TT):
            osb = outsb_pool.tile([128, C], F32, tag="outsb")
            for nch in range(2):
                ps = pj_ps.tile([128, 384], F32, tag="pjps")
                for ct in range(CT):
                    nc.tensor.matmul(
                        ps[:],
                        lhsT=_r(outT[:, ct, tt * 128:(tt + 1) * 128]),
                        rhs=_r(PW[:, ct, nch * 384:(nch + 1) * 384]),
                        start=(ct == 0), stop=(ct == CT - 1))
                nc.vector.tensor_add(osb[:, nch * 384:(nch + 1) * 384], ps[:],
                                     pbb[:, nch * 384:(nch + 1) * 384])
            nc.sync.dma_start(out[tt * 128:(tt + 1) * 128, :], osb[:])
        pj_ctx.close()


_CACHE = {}


def _get_runner():
    """Build + compile once; return a callable(in_maps) -> list of out dicts.

    Keeps a persistent jitted shard_map executable so repeat calls skip
    retracing/recompiling (mirrors bass2jax.run_bass_via_pjrt).
    """
    if "runner" in _CACHE:
        return _CACHE["runner"]

    import jax
    from jax.experimental.shard_map import shard_map
    from jax.sharding import Mesh, PartitionSpec
    from concourse import bass2jax

    nc = _build()
    bass2jax.install_neuronx_cc_hook()

    partition_name = (nc.partition_id_tensor.name if nc.partition_id_tensor
                      else None)
    in_names, out_names, out_avals, zero_outs = [], [], [], []
    for alloc in nc.m.functions[0].allocations:
        if not isinstance(alloc, mybir.MemoryLocationSet):
            continue
        name = alloc.memorylocations[0].name
        if alloc.kind == "ExternalInput":
            if name != partition_name:
                in_names.append(name)
        elif alloc.kind == "ExternalOutput":
            out_names.append(name)
            shape = tuple(alloc.tensor_shape)
            dtype = mybir.dt.np(alloc.dtype)
            out_avals.append(jax.core.ShapedArray(shape, dtype))
            zero_outs.append(np.zeros(shape, dtype))
    n_params = len(in_names)
    n_outs = len(out_avals)
    all_in_names = list(in_names) + list(out_names)
    if partition_name is not None:
        all_in_names.append(partition_name)
    donate = tuple(range(n_params, n_params + n_outs))

    def _body(*args):
        operands = list(args)
        if partition_name is not None:
            operands.append(bass2jax.partition_id_tensor())
        outs = bass2jax._bass_exec_p.bind(
            *operands,
            out_avals=tuple(out_avals),
            in_names=tuple(all_in_names),
            out_names=tuple(out_names),
            lowering_input_output_aliases=(),
            sim_require_finite=True,
            sim_require_nnan=True,
            nc=nc,
        )
        return tuple(outs)

    devices = jax.devices()[:N_CORES]
    mesh = Mesh(np.asarray(devices), ("core",))
    in_specs = (PartitionSpec("core"),) * (n_params + n_outs)
    out_specs = (PartitionSpec("core"),) * n_outs
    sharded = jax.jit(
        shard_map(_body, mesh=mesh, in_specs=in_specs, out_specs=out_specs,
                  check_rep=False),
        donate_argnums=donate, keep_unused=True)

    def runner(in_maps):
        concat_in = [
            np.concatenate([np.asarray(m[name]) for m in in_maps], axis=0)
            for name in in_names
        ]
        concat_zeros = [
            np.zeros((N_CORES * z.shape[0], *z.shape[1:]), z.dtype)
            for z in zero_outs
        ]
        out_arrs = sharded(*concat_in, *concat_zeros)
        return [
            {name: np.asarray(out_arrs[i]).reshape(N_CORES, *out_avals[i].shape)[c]
             for i, name in enumerate(out_names)}
            for c in range(N_CORES)
        ]

    _CACHE["runner"] = runner
    _CACHE["nc"] = nc
    return runner


def _round_f32r(a):
    a = np.ascontiguousarray(a, dtype=np.float32)
    b = a.view(np.uint32) & np.uint32(0xFFFFF800)
    return b.view(np.float32)


def make_in_maps(x, qkv_w, proj_w, proj_b):
    qkv_w = _round_f32r(qkv_w)
    proj_w = _round_f32r(proj_w)
    pb = _round_f32r(np.asarray(proj_b).reshape(1, C))
    return [
        {
            "x_t": _round_f32r(np.asarray(x[b], dtype=np.float32).T),
            "qkv_w": qkv_w,
            "proj_w": proj_w,
            "proj_b": pb,
        }
        for b in range(N_CORES)
    ]


def kernel(x, qkv_w, proj_w, proj_b):
    runner = _get_runner()
    results = runner(make_in_maps(x, qkv_w, proj_w, proj_b))
    return np.stack([results[b]["out"] for b in range(N_CORES)], axis=0)
